# revision 9
# baseline (speedup 1.0000x reference)
"""Causal multi-head attention on 8 trn2 NeuronCores.

Sharding: core = (batch b in {0,1}) x (head-group g in {0..3}; 4 heads each).
QKV weights column-sharded, Wo row-sharded (Megatron TP); each core emits a
partial output for its batch; the host sums the 4 partials per batch and adds
the output bias (the unshard step for row-parallel sharding).

On-chip layout is feature-major: xT (E,S), qT/kT (256,S); v is kept in
natural (s,f) layout padded with a ones column so the AV matmul produces the
softmax denominators for free.  Scores are computed transposed (k,q)
flash-style, so no probability transposes are needed.  All matmuls run in
fp32r.  Softmax skips max-subtraction (scores are O(5); exp is safe in fp32);
causality is enforced with partial-width matmuls plus one 128x128 triangle
mask on diagonal chunks.
"""

import numpy as np

B, S, E, H, D = 2, 2048, 1024, 16, 64
NCORES = 8
G = 4            # head-groups (cores per batch)
HPG = H // G     # heads per core = 4
FS = HPG * D     # feature slice per core = 256
P = 128
QB = 512         # query block (matmul moving width)
NQB = S // QB    # 4
NKC = S // P     # 16 k-chunks

_cache = {}


def _split_waits(nc, mybir, max_waits=1):
    """This walrus build encodes at most one sem-wait per instruction.
    Hoist extra waits onto NOPs inserted before the instruction in the same
    engine stream (same basic block => order preserved)."""
    uid = [0]
    for fn in nc.m.functions:
        for bb in fn.blocks:
            new = []
            changed = False
            for inst in bb.instructions:
                si = inst.sync_info
                if si is not None and len(si.on_wait) > max_waits:
                    waits = list(si.on_wait)
                    head, tail = waits[:-max_waits], waits[-max_waits:]
                    for k in range(0, len(head), max_waits):
                        nop = mybir.InstNoOp(name=f"WSPLIT-{uid[0]}", ins=[], outs=[])
                        uid[0] += 1
                        nop.engine = inst.engine
                        nop.sync_info = mybir.SyncInfo(
                            on_wait=head[k:k + max_waits], on_update=[])
                        new.append(nop)
                    inst.sync_info = mybir.SyncInfo(
                        on_wait=tail, on_update=list(si.on_update))
                    changed = True
                new.append(inst)
            if changed:
                bb.instructions = new


def _build():
    if "nc" in _cache:
        return _cache["nc"]

    import concourse.bass as bass
    import concourse.mybir as mybir
    import concourse.tile as tile

    F32 = mybir.dt.float32
    F32R = mybir.dt.float32r
    EXP = mybir.ActivationFunctionType.Exp

    nc = bass.Bass("TRN2", target_bir_lowering=False, debug=False)

    xt_d = nc.dram_tensor("xt", [E, S], F32R, kind="ExternalInput")
    wq_d = nc.dram_tensor("wqt", [E, FS], F32R, kind="ExternalInput")
    wk_d = nc.dram_tensor("wkt", [E, FS], F32R, kind="ExternalInput")
    wv_d = nc.dram_tensor("wvt", [E, FS], F32R, kind="ExternalInput")
    wo_d = nc.dram_tensor("wot", [FS, E], F32R, kind="ExternalInput")
    bq_d = nc.dram_tensor("bq", [2, P], F32, kind="ExternalInput")
    bk_d = nc.dram_tensor("bk", [2, P], F32, kind="ExternalInput")
    bv_d = nc.dram_tensor("bvb", [P, FS], F32, kind="ExternalInput")   # pre-broadcast
    mask_d = nc.dram_tensor("mask", [P, P], F32R, kind="ExternalInput")  # tri: 1 if j>=k
    out_d = nc.dram_tensor("outt", [E, S], F32, kind="ExternalOutput")

    EC = E // P  # 8 contraction chunks for projections

    with tile.TileContext(nc) as tc, \
         nc.allow_low_precision(reason="fp32r rounding for PE operands is intended"):
        with tc.tile_pool(name="big", bufs=1) as big, \
             tc.tile_pool(name="small", bufs=1) as small:

            # ---- resident inputs ----
            xt = []
            for c in range(EC):
                t = big.tile([P, S], F32R, name=f"xt{c}", tag=f"xt{c}")
                nc.sync.dma_start(t[:], xt_d[bass.ts(c, P), :])
                xt.append(t)
            wq_t = big.tile([P, EC, FS], F32R, tag="wq")
            nc.sync.dma_start(wq_t[:], wq_d.rearrange("(c p) f -> p c f", p=P))
            wk_t = big.tile([P, EC, FS], F32R, tag="wk")
            nc.sync.dma_start(wk_t[:], wk_d.rearrange("(c p) f -> p c f", p=P))
            wv_t = big.tile([P, EC, FS], F32R, tag="wv")
            nc.sync.dma_start(wv_t[:], wv_d.rearrange("(c p) f -> p c f", p=P))
            wo_t = big.tile([P, 2, E], F32R, tag="wo")
            nc.sync.dma_start(wo_t[:], wo_d.rearrange("(c p) e -> p c e", p=P))
            bq_t = small.tile([P, 2], F32, tag="bq")
            nc.sync.dma_start(bq_t[:], bq_d.rearrange("c p -> p c"))
            bk_t = small.tile([P, 2], F32, tag="bk")
            nc.sync.dma_start(bk_t[:], bk_d.rearrange("c p -> p c"))
            bv_t = small.tile([P, FS], F32, tag="bv")
            nc.sync.dma_start(bv_t[:], bv_d[:])
            mask_t = small.tile([P, P], F32R, tag="mask")
            nc.sync.dma_start(mask_t[:], mask_d[:])
            ones_f = small.tile([P, D], F32, tag="onesf")
            nc.any.memset(ones_f[:], 1.0)
            ones_r = small.tile([1, D], F32R, tag="onesr")
            nc.vector.tensor_copy(ones_r[:], ones_f[0:1, :])

            # ---- outputs of phase 1 (resident) ----
            qT = [big.tile([P, S], F32R, name=f"qT{f}", tag=f"qT{f}") for f in range(2)]
            kT = [big.tile([P, S], F32R, name=f"kT{f}", tag=f"kT{f}") for f in range(2)]
            vpad = [big.tile([P, HPG, D + 1], F32R, name=f"vp{c}", tag=f"vp{c}") for c in range(NKC)]
            attnT = [big.tile([P, S], F32R, name=f"aT{f}", tag=f"aT{f}") for f in range(2)]

            # ---- phase 1: projections ----
            with tc.tile_pool(name="pproj", bufs=4, space="PSUM") as pp:
                for fc in range(2):
                    for sc in range(NQB):
                        for dst, w, bias in ((qT, wq_t, bq_t), (kT, wk_t, bk_t)):
                            ps = pp.tile([P, QB], F32, tag="pqk")
                            for ec in range(EC):
                                nc.tensor.matmul(
                                    ps[:], w[:, ec, bass.ts(fc, P)],
                                    xt[ec][:, bass.ts(sc, QB)],
                                    start=(ec == 0), stop=(ec == EC - 1))
                            nc.vector.tensor_add(
                                dst[fc][:, bass.ts(sc, QB)], ps[:],
                                bias[:, fc:fc + 1].to_broadcast((P, QB)))
                for sc in range(NKC):
                    ps = pp.tile([P, FS], F32, tag="pv")
                    for ec in range(EC):
                        nc.tensor.matmul(
                            ps[:], xt[ec][:, bass.ts(sc, P)], wv_t[:, ec, :],
                            start=(ec == 0), stop=(ec == EC - 1))
                    psv = ps.rearrange("p (h d) -> p h d", h=HPG)
                    bvv = bv_t.rearrange("p (h d) -> p h d", h=HPG)
                    nc.vector.tensor_add(vpad[sc][:, :, 0:D], psv[:], bvv[:])
                    nc.vector.tensor_copy(vpad[sc][:, :, D:D + 1],
                                          ones_f[:, 0:HPG][:, :, None])

            # ---- phase 2: attention per (head, q-block) ----
            with tc.tile_pool(name="psc", bufs=3, space="PSUM") as psc, \
                 tc.tile_pool(name="pav", bufs=2, space="PSUM") as pav, \
                 tc.tile_pool(name="prb", bufs=2, space="PSUM") as prb, \
                 tc.tile_pool(name="pt", bufs=3) as ptp, \
                 tc.tile_pool(name="rc", bufs=2) as rcp:
                for h in range(HPG):
                    fc, ro = h // 2, (h % 2) * D
                    for qb in range(NQB):
                        q0 = qb * QB
                        nchunks = (q0 + QB) // P
                        av = pav.tile([D + 1, QB], F32, tag="av")
                        pend = None  # (pT, delta, chunk)
                        for c in range(nchunks):
                            delta = max(0, c * P - q0)
                            sp = psc.tile([P, QB], F32, tag="sc")
                            nc.tensor.matmul(
                                sp[:, delta:QB],
                                kT[fc][ro:ro + D, bass.ts(c, P)],
                                qT[fc][ro:ro + D, q0 + delta:q0 + QB],
                                start=True, stop=True)
                            if pend is not None:
                                pT0, d0, c0 = pend
                                nc.tensor.matmul(
                                    av[:, d0:QB], vpad[c0][:, h, :], pT0[:, d0:QB],
                                    start=(c0 == 0), stop=False)
                            pT = ptp.tile([P, QB], F32R, tag="pT")
                            nc.scalar.activation(
                                pT[:, delta:QB], sp[:, delta:QB], EXP, scale=0.125)
                            if c * P >= q0:
                                nc.vector.tensor_mul(
                                    pT[:, delta:delta + P],
                                    pT[:, delta:delta + P], mask_t[:])
                            pend = (pT, delta, c)
                        pT0, d0, c0 = pend
                        nc.tensor.matmul(
                            av[:, d0:QB], vpad[c0][:, h, :], pT0[:, d0:QB],
                            start=(c0 == 0), stop=True)
                        recip = rcp.tile([1, QB], F32R, tag="recip")
                        nc.vector.reciprocal(recip[:], av[D:D + 1, :])
                        rb = prb.tile([D, QB], F32, tag="rb")
                        nc.tensor.matmul(rb[:], ones_r[:], recip[:],
                                         start=True, stop=True)
                        rbs = rcp.tile([D, QB], F32, tag="rbs")
                        nc.vector.tensor_copy(rbs[:], rb[:])
                        nc.vector.tensor_mul(
                            attnT[fc][ro:ro + D, q0:q0 + QB], av[0:D, :], rbs[:])

            # ---- phase 3: output projection (partial) ----
            with tc.tile_pool(name="po", bufs=4, space="PSUM") as pop, \
                 tc.tile_pool(name="ot", bufs=3) as otp:
                for m in range(EC):
                    for sc in range(NQB):
                        ps = pop.tile([P, QB], F32, tag="po")
                        nc.tensor.matmul(ps[:], wo_t[:, 0, bass.ts(m, P)],
                                         attnT[0][:, bass.ts(sc, QB)],
                                         start=True, stop=False)
                        nc.tensor.matmul(ps[:], wo_t[:, 1, bass.ts(m, P)],
                                         attnT[1][:, bass.ts(sc, QB)],
                                         start=False, stop=True)
                        ot = otp.tile([P, QB], F32, tag="ot")
                        nc.vector.tensor_copy(ot[:], ps[:])
                        nc.sync.dma_start(
                            out_d[bass.ts(m, P), bass.ts(sc, QB)], ot[:])

    _split_waits(nc, mybir)
    _cache["nc"] = nc
    return nc


def _in_maps(x, Wq, bq, Wk, bk, Wv, bv, Wo, bo):
    f32 = np.float32
    xT = [np.ascontiguousarray(x[b].T, dtype=f32) for b in range(B)]
    WqT = np.ascontiguousarray(Wq.T, dtype=f32)
    WkT = np.ascontiguousarray(Wk.T, dtype=f32)
    WvT = np.ascontiguousarray(Wv.T, dtype=f32)
    # out = attn @ Wo.T -> partial over feature slice: lhsT rows = local f
    tri = np.triu(np.ones((P, P), dtype=f32))  # [k, j] = 1 if j >= k
    maps = []
    for core in range(NCORES):
        b, g = divmod(core, G)
        fs = slice(g * FS, (g + 1) * FS)
        maps.append({
            "xt": xT[b],
            "wqt": np.ascontiguousarray(WqT[:, fs]),
            "wkt": np.ascontiguousarray(WkT[:, fs]),
            "wvt": np.ascontiguousarray(WvT[:, fs]),
            "wot": np.ascontiguousarray(Wo[:, fs].T),
            "bq": np.ascontiguousarray(bq[fs].reshape(2, P)),
            "bk": np.ascontiguousarray(bk[fs].reshape(2, P)),
            "bvb": np.broadcast_to(bv[fs], (P, FS)).copy(),
            "mask": tri,
        })
    return maps


def _runner():
    """Compile once; return (exec_fn, put_fn).

    put_fn(maps) -> device args (inputs resident on the 8 cores).
    exec_fn(args) -> list of 8 per-core output dicts (numpy).
    """
    if "run" in _cache:
        return _cache["run"]

    import jax
    from jax.experimental.shard_map import shard_map
    from jax.sharding import Mesh, NamedSharding, PartitionSpec

    import concourse.mybir as mybir
    from concourse.bass2jax import (
        _bass_exec_p,
        install_neuronx_cc_hook,
        partition_id_tensor,
    )

    nc = _build()
    install_neuronx_cc_hook()

    partition_name = nc.partition_id_tensor.name if nc.partition_id_tensor else None
    in_names, out_names, out_avals, zero_outs = [], [], [], []
    for alloc in nc.m.functions[0].allocations:
        if not isinstance(alloc, mybir.MemoryLocationSet):
            continue
        name = alloc.memorylocations[0].name
        if alloc.kind == "ExternalInput":
            if name != partition_name:
                in_names.append(name)
        elif alloc.kind == "ExternalOutput":
            shape = tuple(alloc.tensor_shape)
            dtype = mybir.dt.np(alloc.dtype)
            out_names.append(name)
            out_avals.append(jax.core.ShapedArray(shape, dtype))
            zero_outs.append(np.zeros(shape, dtype))
    n_params = len(in_names)
    all_in_names = list(in_names) + list(out_names)
    if partition_name is not None:
        all_in_names.append(partition_name)

    def _body(*args):
        operands = list(args)
        if partition_name is not None:
            operands.append(partition_id_tensor())
        outs = _bass_exec_p.bind(
            *operands,
            out_avals=tuple(out_avals),
            in_names=tuple(all_in_names),
            out_names=tuple(out_names),
            lowering_input_output_aliases=(),
            sim_require_finite=True,
            sim_require_nnan=True,
            nc=nc,
        )
        return tuple(outs)

    devices = jax.devices()[:NCORES]
    mesh = Mesh(np.asarray(devices), ("core",))
    n_ops = n_params + len(out_names)
    sharded = jax.jit(
        shard_map(
            _body, mesh=mesh,
            in_specs=(PartitionSpec("core"),) * n_ops,
            out_specs=(PartitionSpec("core"),) * len(out_names),
            check_rep=False,
        ),
        keep_unused=True,
    )
    shard = NamedSharding(mesh, PartitionSpec("core"))

    def put_fn(maps):
        concat = [
            np.concatenate([np.asarray(maps[c][n]) for c in range(NCORES)], axis=0)
            for n in in_names
        ] + [
            np.concatenate([z] * NCORES, axis=0) for z in zero_outs
        ]
        return [jax.device_put(a, shard) for a in concat]

    def exec_fn(args):
        out_arrs = sharded(*args)
        jax.block_until_ready(out_arrs)
        return [
            {
                n: np.asarray(out_arrs[i]).reshape(NCORES, *out_avals[i].shape)[c]
                for i, n in enumerate(out_names)
            }
            for c in range(NCORES)
        ]

    _cache["run"] = (exec_fn, put_fn)
    return _cache["run"]


def _assemble(results, bo):
    out = np.empty((B, S, E), dtype=np.float32)
    for b in range(B):
        acc = results[b * G]["outt"].astype(np.float32)
        for g in range(1, G):
            acc = acc + results[b * G + g]["outt"]
        out[b] = acc.T + bo
    return out


def kernel(x, Wq, bq, Wk, bk, Wv, bv, Wo, bo):
    exec_fn, put_fn = _runner()
    maps = _in_maps(x, Wq, bq, Wk, bk, Wv, bv, Wo, bo)
    args = put_fn(maps)
    if not _cache.get("warm"):
        # First execution after load can race device-side initialization;
        # run once and discard, then use the steady-state result.
        exec_fn(args)
        _cache["warm"] = True
    results = exec_fn(args)
    return _assemble(results, bo)
